# Initial kernel scaffold
#
"""Trainium2 Bass kernel for CrossLevelAttention (dense_transformer).

Math: the reference's _mha_seq1 with seq_len=1 is affine:
    upd = (x @ Wv.T + bv) @ Wout.T + bout = x @ (Wout

@Wv).T + (Wout@bv + bout)
so each of the 4 attention blocks collapses to ONE [B,D]x[D,D] matmul with
host-precombined weights M_i = Wout_i @ Wv_i and bias c_i = Wout_i@bv_i + bout_i.

Device kernel (per core, data-parallel over batch, 8192 rows each):
  for each 128-row tile:
    block(i, x, res, ln?):  u = x @ M_i.T  (PE, fp16 operands, fp32 PSUM accum)
                            y = u + res + c_i
                            out = LN(y)*g+b (blocks 1-3) or y (block 4)
  chain: tac1 = LN1(tactical + B1(local)); strat = LN2(strategic + B2(tac1));
         tac2 = LN3(tac1 + B3(strat));    loc  = local + B4(tac2)
  outputs: (loc, tac2, strat)

Row-major layout (rows on partitions); the matmul stationary operand is the
PE-transposed activation chunk; M_i.T chunks stream as the moving operand.
"""

import os
import sys
from contextlib import ExitStack

import numpy as np

import concourse.bass as bass
import concourse.tile as tile
from concourse import bacc, mybir
from concourse.bass_utils import run_bass_kernel_spmd
from concourse.masks import make_identity

B = 65536
D = 1024
EPS = 1e-5
NCORES = 8
ROWS_PER_CORE = B // NCORES  # 8192
P = 128

# matmul operand dtype (weights + transposed activations). fp16 keeps ~2^-11
# relative precision on this data (|x| <= ~6, |M| ~ 0.02) at full PE rate.
DT_MM = mybir.dt.float16
F32 = mybir.dt.float32

KCH = D // P  # 8 contraction chunks
NHALF = 512  # matmul free dim per PSUM bank (fp32 out)


def build_kernel(rows_per_core=ROWS_PER_CORE, debug=False):
    """Build the Bass module. Returns (nc, meta) where meta maps names."""
    nc = bacc.Bacc(
        "TRN2",
        target_bir_lowering=False,
        debug=debug,
        enable_asserts=False,
        num_devices=NCORES,
    )

    dram = {}
    for name in ("x_loc", "x_tac", "x_str"):
        dram[name] = nc.dram_tensor(
            name, [rows_per_core, D], F32, kind="ExternalInput"
        ).ap()
    for i in range(4):
        dram[f"mt{i}"] = nc.dram_tensor(
            f"mt{i}", [D, D], F32, kind="ExternalInput"
        ).ap()
        dram[f"c{i}"] = nc.dram_tensor(f"c{i}", [D], F32, kind="ExternalInput").ap()
    for j in (1, 2, 3):
        dram[f"g{j}"] = nc.dram_tensor(f"g{j}", [D], F32, kind="ExternalInput").ap()
        dram[f"b{j}"] = nc.dram_tensor(f"b{j}", [D], F32, kind="ExternalInput").ap()
    for name in ("o_loc", "o_tac", "o_str"):
        dram[name] = nc.dram_tensor(
            name, [rows_per_core, D], F32, kind="ExternalOutput"
        ).ap()

    with tile.TileContext(nc) as tc:
        with ExitStack() as ctx:
            _kernel_body(ctx, tc, dram, rows_per_core)

    nc.compile()
    return nc


def _bcast_row(vec_ap):
    """AP reading a [D] dram vector replicated across 128 partitions."""
    return bass.AP(
        tensor=vec_ap.tensor, offset=vec_ap.offset, ap=[[0, P]] + list(vec_ap.ap)
    )


def _kernel_body(ctx, tc, dram, rows_per_core):
    nc = tc.nc
    ntiles = rows_per_core // P

    consts = ctx.enter_context(tc.tile_pool(name="consts", bufs=1))
    wpool = ctx.enter_context(tc.tile_pool(name="weights", bufs=1))
    iopool = ctx.enter_context(tc.tile_pool(name="io", bufs=2))
    actpool = ctx.enter_context(tc.tile_pool(name="act", bufs=2))
    scratch = ctx.enter_context(tc.tile_pool(name="scratch", bufs=4))
    stats = ctx.enter_context(tc.tile_pool(name="stats", bufs=4))
    psum_v = ctx.enter_context(tc.tile_pool(name="psum_v", bufs=2, space="PSUM"))
    psum_t = ctx.enter_context(tc.tile_pool(name="psum_t", bufs=2, space="PSUM"))

    # ---- constants ----
    identity = consts.tile([P, P], DT_MM, tag="identity")
    make_identity(nc, identity)
    eps_t = consts.tile([P, 1], F32, tag="eps")
    nc.vector.memset(eps_t, EPS)

    cb = []
    for i in range(4):
        t = consts.tile([P, D], F32, tag=f"cb{i}")
        nc.sync.dma_start(out=t, in_=_bcast_row(dram[f"c{i}"]))
        cb.append(t)
    gb = {}
    for j in (1, 2, 3):
        tg = consts.tile([P, D], F32, tag=f"g{j}")
        nc.sync.dma_start(out=tg, in_=_bcast_row(dram[f"g{j}"]))
        tb = consts.tile([P, D], F32, tag=f"b{j}")
        nc.sync.dma_start(out=tb, in_=_bcast_row(dram[f"b{j}"]))
        gb[j] = (tg, tb)

    # ---- weights: load fp32, convert to DT_MM, keep resident ----
    mt = []
    for i in range(4):
        wt = wpool.tile([P, KCH, D], DT_MM, tag=f"mt{i}")
        for k in range(KCH):
            stg = scratch.tile([P, D], F32, tag="wstage")
            nc.sync.dma_start(out=stg, in_=dram[f"mt{i}"][k * P : (k + 1) * P, :])
            nc.gpsimd.tensor_copy(out=wt[:, k], in_=stg)
        mt.append(wt)

    # ---- helpers ----
    def matmul_block(i, x_f32):
        """u = x @ M_i.T ; x_f32 is [P, D] fp32 sbuf. Returns PSUM [P, D] f32."""
        xb = scratch.tile([P, D], DT_MM, tag="xb")
        nc.scalar.copy(out=xb, in_=x_f32)
        xt = scratch.tile([P, KCH, P], DT_MM, tag="xt")
        for k in range(KCH):
            pt = psum_t.tile([P, P], DT_MM, tag="pt")
            nc.tensor.transpose(pt, xb[:, k * P : (k + 1) * P], identity)
            nc.scalar.copy(out=xt[:, k], in_=pt)
        v = psum_v.tile([P, D], F32, tag="v")
        for k in range(KCH):
            for h in range(D // NHALF):
                nc.tensor.matmul(
                    v[:, h * NHALF : (h + 1) * NHALF],
                    lhsT=xt[:, k],
                    rhs=mt[i][:, k, h * NHALF : (h + 1) * NHALF],
                    start=(k == 0),
                    stop=(k == KCH - 1),
                )
        return v

    def layernorm(y, j, out_tile):
        """out = LN(y)*g+b. y is [P, D] f32 sbuf."""
        st = stats.tile([P, 2, 6], F32, tag="bnst")
        nc.vector.bn_stats(st[:, 0], y[:, :512])
        nc.vector.bn_stats(st[:, 1], y[:, 512:])
        mv = stats.tile([P, 2], F32, tag="mv")
        nc.vector.bn_aggr(mv, st)
        std = stats.tile([P, 1], F32, tag="std")
        nc.scalar.activation(
            out=std,
            in_=mv[:, 1:2],
            func=mybir.ActivationFunctionType.Sqrt,
            bias=eps_t,
        )
        rstd = stats.tile([P, 1], F32, tag="rstd")
        nc.vector.reciprocal(rstd, std)
        norm = scratch.tile([P, D], F32, tag="norm")
        nc.vector.tensor_scalar(
            norm,
            y,
            mv[:, 0:1],
            rstd,
            mybir.AluOpType.subtract,
            mybir.AluOpType.mult,
        )
        tg, tb = gb[j]
        t2 = scratch.tile([P, D], F32, tag="t2")
        nc.gpsimd.tensor_mul(t2, norm, tg)
        nc.gpsimd.tensor_add(out_tile, t2, tb)

    # ---- main loop ----
    for t in range(ntiles):
        rows = slice(t * P, (t + 1) * P)
        x_loc = iopool.tile([P, D], F32, tag="x_loc")
        nc.sync.dma_start(out=x_loc, in_=dram["x_loc"][rows, :])
        x_tac = iopool.tile([P, D], F32, tag="x_tac")
        nc.sync.dma_start(out=x_tac, in_=dram["x_tac"][rows, :])
        x_str = iopool.tile([P, D], F32, tag="x_str")
        nc.sync.dma_start(out=x_str, in_=dram["x_str"][rows, :])

        # block 1: tac1 = LN1(tactical + local@M1.T + c1)
        v1 = matmul_block(0, x_loc)
        tmp = scratch.tile([P, D], F32, tag="tmp")
        nc.gpsimd.tensor_add(tmp, x_tac, cb[0])
        y1 = scratch.tile([P, D], F32, tag="y")
        nc.vector.tensor_add(y1, v1, tmp)
        tac1 = actpool.tile([P, D], F32, tag="tac1")
        layernorm(y1, 1, tac1)

        # block 2: strat = LN2(strategic + tac1@M2.T + c2)
        v2 = matmul_block(1, tac1)
        tmp = scratch.tile([P, D], F32, tag="tmp")
        nc.gpsimd.tensor_add(tmp, x_str, cb[1])
        y2 = scratch.tile([P, D], F32, tag="y")
        nc.vector.tensor_add(y2, v2, tmp)
        strat = actpool.tile([P, D], F32, tag="strat")
        layernorm(y2, 2, strat)
        nc.sync.dma_start(out=dram["o_str"][rows, :], in_=strat)

        # block 3: tac2 = LN3(tac1 + strat@M3.T + c3)
        v3 = matmul_block(2, strat)
        tmp = scratch.tile([P, D], F32, tag="tmp")
        nc.gpsimd.tensor_add(tmp, tac1, cb[2])
        y3 = scratch.tile([P, D], F32, tag="y")
        nc.vector.tensor_add(y3, v3, tmp)
        tac2 = actpool.tile([P, D], F32, tag="tac2")
        layernorm(y3, 3, tac2)
        nc.sync.dma_start(out=dram["o_tac"][rows, :], in_=tac2)

        # block 4 (no LN): loc = local + tac2@M4.T + c4
        v4 = matmul_block(3, tac2)
        tmp = scratch.tile([P, D], F32, tag="tmp")
        nc.gpsimd.tensor_add(tmp, x_loc, cb[3])
        loc = actpool.tile([P, D], F32, tag="loc")
        nc.vector.tensor_add(loc, v4, tmp)
        nc.sync.dma_start(out=dram["o_loc"][rows, :], in_=loc)


def host_prep(inputs, rows_per_core=ROWS_PER_CORE, ncores=NCORES):
    """Slice/precombine weights on host, shard batch across cores."""
    f = np.float32
    per_core_common = {}
    for i, name in enumerate(("l2t", "t2s", "s2t", "t2l")):
        w_in = np.asarray(inputs[f"{name}_w_in"], f)
        b_in = np.asarray(inputs[f"{name}_b_in"], f)
        w_out = np.asarray(inputs[f"{name}_w_out"], f)
        b_out = np.asarray(inputs[f"{name}_b_out"], f)
        wv = w_in[2 * D :]  # [D, D]
        bv = b_in[2 * D :]  # [D]
        m = w_out @ wv  # [D, D]
        per_core_common[f"mt{i}"] = np.ascontiguousarray(m.T)
        per_core_common[f"c{i}"] = np.ascontiguousarray(w_out @ bv + b_out)
    for j in (1, 2, 3):
        per_core_common[f"g{j}"] = np.asarray(inputs[f"ln{j}_g"], f)
        per_core_common[f"b{j}"] = np.asarray(inputs[f"ln{j}_b"], f)

    loc = np.asarray(inputs["local_features"], f)
    tac = np.asarray(inputs["tactical_features"], f)
    stra = np.asarray(inputs["strategic_features"], f)
    in_maps = []
    for c in range(ncores):
        rows = slice(c * rows_per_core, (c + 1) * rows_per_core)
        m = dict(per_core_common)
        m["x_loc"] = np.ascontiguousarray(loc[rows])
        m["x_tac"] = np.ascontiguousarray(tac[rows])
        m["x_str"] = np.ascontiguousarray(stra[rows])
        in_maps.append(m)
    return in_maps


_CACHED = {}


def _get_compiled():
    if "nc" not in _CACHED:
        _CACHED["nc"] = build_kernel()
    return _CACHED["nc"]


def kernel(**inputs):
    nc = _get_compiled()
    in_maps = host_prep(inputs)
    res = run_bass_kernel_spmd(nc, in_maps, list(range(NCORES)))
    loc = np.concatenate([res.results[c]["o_loc"] for c in range(NCORES)], axis=0)
    tac = np.concatenate([res.results[c]["o_tac"] for c in range(NCORES)], axis=0)
    stra = np.concatenate([res.results[c]["o_str"] for c in range(NCORES)], axis=0)
    return (loc, tac, stra)


# revision 32
# speedup vs baseline: 3.9812x; 3.9812x over previous
"""Trainium2 Bass kernel for CrossLevelAttention (dense_transformer).

Math: the reference's _mha_seq1 with seq_len=1 is affine:
    upd = (x @ Wv.T + bv) @ Wout.T + bout = x @ (Wout@Wv).T + (Wout@bv + bout)
so each of the 4 attention blocks collapses to ONE [B,D]x[D,D] matmul with
host-precombined weights M_i = Wout_i @ Wv_i and bias c_i = Wout_i@bv_i + bout_i.

Device kernel (per core, data-parallel over batch, 8192 rows each), per
128-row tile, per block i:
    v   = c_i (K=1 ones-matmul) + x @ M_i.T (8 fp16 matmuls) + res (fp32r
          identity-matmul)        -- all accumulated in one PSUM group
    out = LN(v)*g+b (blocks 1-3, bn_stats + 2 fused scalar_tensor_tensor)
          or copy(v) (block 4)
chain: tac1 = LN1(tactical + B1(local)); strat = LN2(strategic + B2(tac1));
       tac2 = LN3(tac1 + B3(strat));    loc  = local + B4(tac2)
outputs: (loc, tac2, strat)

Row-major layout (rows on partitions). The matmul stationary operand is the
DMA-transposed (fp16, 2-byte XBAR path) activation; M_i.T chunks stream as
the moving operand.
"""

from contextlib import ExitStack

import numpy as np

import concourse.bass as bass
import concourse.tile as tile
from concourse import bacc, mybir
from concourse.bass_utils import run_bass_kernel_spmd
from concourse.masks import make_identity

B = 65536
D = 1024
EPS = 1e-5
NCORES = 8
ROWS_PER_CORE = B // NCORES  # 8192
P = 128

DT_MM = mybir.dt.float16  # matmul operand dtype (weights + transposed acts)
F32 = mybir.dt.float32

KCH = D // P  # 8 contraction chunks
NHALF = 512  # matmul free dim per PSUM bank (fp32 out)
HALVES = D // NHALF


def build_kernel(rows_per_core=ROWS_PER_CORE, debug=False, passes=1):
    nc = bacc.Bacc(
        "TRN2",
        target_bir_lowering=False,
        debug=debug,
        enable_asserts=False,
        num_devices=NCORES,
    )

    dram = {}
    for name in ("x_loc", "x_tac", "x_str"):
        dram[name] = nc.dram_tensor(
            name, [rows_per_core, D], F32, kind="ExternalInput"
        ).ap()
    for i in range(4):
        dram[f"mt{i}"] = nc.dram_tensor(
            f"mt{i}", [D, D], F32, kind="ExternalInput"
        ).ap()
        dram[f"c{i}"] = nc.dram_tensor(f"c{i}", [D], F32, kind="ExternalInput").ap()
    for j in (1, 2, 3):
        dram[f"g{j}"] = nc.dram_tensor(f"g{j}", [D], F32, kind="ExternalInput").ap()
        dram[f"b{j}"] = nc.dram_tensor(f"b{j}", [D], F32, kind="ExternalInput").ap()
    for name in ("o_loc", "o_tac", "o_str"):
        dram[name] = nc.dram_tensor(
            name, [rows_per_core, D], F32, kind="ExternalOutput"
        ).ap()

    with tile.TileContext(nc) as tc:
        with ExitStack() as ctx:
            _kernel_body(ctx, tc, dram, rows_per_core, passes=passes)

    nc.compile()
    return nc


def _bcast_row(vec_ap, parts=P):
    return bass.AP(
        tensor=vec_ap.tensor, offset=vec_ap.offset, ap=[[0, parts]] + list(vec_ap.ap)
    )


def _kernel_body(ctx, tc, dram, rows_per_core, passes=1):
    nc = tc.nc
    ntiles = rows_per_core // P

    consts = ctx.enter_context(tc.tile_pool(name="consts", bufs=1))
    wpool = ctx.enter_context(tc.tile_pool(name="weights", bufs=1))
    iopool = ctx.enter_context(tc.tile_pool(name="io", bufs=4))
    iopool3 = ctx.enter_context(tc.tile_pool(name="io3", bufs=3))
    actpool = ctx.enter_context(tc.tile_pool(name="act", bufs=3))
    outpool = ctx.enter_context(tc.tile_pool(name="out", bufs=2))
    xtpool = ctx.enter_context(tc.tile_pool(name="xtp", bufs=6))
    xbpool = ctx.enter_context(tc.tile_pool(name="xbp", bufs=6))
    scratch = ctx.enter_context(tc.tile_pool(name="scratch", bufs=3))
    tpool = ctx.enter_context(tc.tile_pool(name="tp", bufs=2))
    xstage = ctx.enter_context(tc.tile_pool(name="xstage", bufs=3))
    stats = ctx.enter_context(tc.tile_pool(name="stats", bufs=6))
    psum_v = ctx.enter_context(tc.tile_pool(name="psum_v", bufs=8, space="PSUM"))

    # ---- constants ----
    identity = consts.tile([P, P], DT_MM, tag="identity")
    make_identity(nc, identity)
    ones_col = consts.tile([1, P], DT_MM, tag="ones_col")
    nc.vector.memset(ones_col, 1.0)
    eps_t = consts.tile([P, 1], F32, tag="eps")
    nc.vector.memset(eps_t, EPS)

    def _bcast_f16(vec_ap, tag):
        stg = scratch.tile([P, D], F32, tag="t")
        nc.sync.dma_start(out=stg, in_=_bcast_row(vec_ap))
        t = consts.tile([P, D], DT_MM, tag=tag)
        nc.vector.tensor_copy(out=t, in_=stg)
        return t

    c_all = consts.tile([1, 4, D], DT_MM, tag="c_all")
    for i in range(4):
        stg = scratch.tile([P, D], F32, tag="t")
        nc.sync.dma_start(out=stg[0:1, :], in_=_bcast_row(dram[f"c{i}"], parts=1))
        nc.vector.tensor_copy(out=c_all[:, i, :], in_=stg[0:1, :])
    c_row = [c_all[:, i, :] for i in range(4)]
    gb = {
        j: (_bcast_f16(dram[f"g{j}"], f"g{j}"), _bcast_f16(dram[f"b{j}"], f"b{j}"))
        for j in (1, 2, 3)
    }

    # ---- weights: load fp32, convert to fp16, keep resident.
    # mt0 loads up front; mt1-3 stream in during the first iterations so
    # tile 0's compute doesn't wait behind 32 weight DMAs.
    mt = [wpool.tile([P, KCH, D], DT_MM, tag=f"mt{i}", name=f"mt{i}") for i in range(4)]

    def load_weights(i):
        for k in range(KCH):
            stg = outpool.tile([P, D], F32, tag="loc", name=f"ws{i}_{k}")
            nc.sync.dma_start(out=stg, in_=dram[f"mt{i}"][k * P : (k + 1) * P, :])
            nc.vector.tensor_copy(out=mt[i][:, k], in_=stg)

    def alloc_xb(name):
        return xbpool.tile([P, D], DT_MM, tag="xb", name=f"xb_{name}")

    def transpose_xb(xb, name):
        xt = xtpool.tile([P, KCH, P], DT_MM, tag="xt", name=f"xt_{name}")
        nc.sync.dma_start_transpose(out=xt, in_=xb)
        return xt

    def matmul_block(i, xt, res_f16, name):
        """PSUM halves v[h] = c_i + x @ M_i.T + res (one group per bank)."""
        v = [
            psum_v.tile([P, NHALF], F32, tag="v", name=f"v_{name}_{h}")
            for h in range(HALVES)
        ]
        # half 0 fully first so its stats can overlap half 1's matmuls
        for h in range(HALVES):
            hs = slice(h * NHALF, (h + 1) * NHALF)
            nc.tensor.matmul(
                v[h], lhsT=ones_col, rhs=c_row[i][:, hs], start=True, stop=False
            )
            for k in range(KCH):
                nc.tensor.matmul(
                    v[h], lhsT=xt[:, k], rhs=mt[i][:, k, hs], start=False, stop=False
                )
            nc.tensor.matmul(
                v[h],
                lhsT=identity,
                rhs=res_f16[:, hs],
                start=False,
                stop=True,
            )
        return v

    def layernorm(v, j, out_tile, xb_out=None):
        """out = LN(v)*g+b; v is a list of [P, NHALF] f32 PSUM halves.

        If xb_out is given, the fp16 copy (next block's matmul input) is
        written FIRST by a twin fused op, so the transpose path doesn't
        wait behind the fp32 write.
        """
        st = stats.tile([P, HALVES, 6], F32, tag="bnst")
        for h in range(HALVES):
            nc.vector.bn_stats(st[:, h], v[h])
        mv = stats.tile([P, 2], F32, tag="mv")
        nc.vector.bn_aggr(mv, st)
        std = stats.tile([P, 1], F32, tag="std")
        nc.scalar.activation(
            out=std,
            in_=mv[:, 1:2],
            func=mybir.ActivationFunctionType.Sqrt,
            bias=eps_t,
        )
        rstd = stats.tile([P, 1], F32, tag="rstd")
        nc.vector.reciprocal(rstd, std)
        tg, tb = gb[j]
        t = tpool.tile([P, D], F32, tag="t")
        for h in range(HALVES):
            hs = slice(h * NHALF, (h + 1) * NHALF)
            # t = (v - mean) * g   (releases the PSUM bank)
            nc.vector.scalar_tensor_tensor(
                out=t[:, hs],
                in0=v[h],
                scalar=mv[:, 0:1],
                in1=tg[:, hs],
                op0=mybir.AluOpType.subtract,
                op1=mybir.AluOpType.mult,
            )
        # out = t * rstd + b
        dsts = ([xb_out] if xb_out is not None else []) + (
            [out_tile] if out_tile is not None else []
        )
        for dst in dsts:
            nc.vector.scalar_tensor_tensor(
                out=dst,
                in0=t,
                scalar=rstd,
                in1=tb,
                op0=mybir.AluOpType.mult,
                op1=mybir.AluOpType.add,
            )

    # ---- software-pipelined main loop ----
    # Tile `a` runs phase A (block1+block2) while tile `b = a-1` runs
    # phase B (block3+block4). Per iteration, PE groups are emitted first
    # (their inputs were produced in earlier iterations), so every
    # in-order engine queue always has ready work at its head.
    S = {}  # per-tile pipeline state

    def stage_load(i):
        rows = slice((i % ntiles) * P, (i % ntiles + 1) * P)
        st = {}
        for nm in ("x_loc", "x_tac", "x_str"):
            pool = iopool if nm == "x_loc" else iopool3
            stg = xstage.tile([P, D], F32, tag="xs", name=f"xs_{nm}_{i}")
            nc.sync.dma_start(out=stg, in_=dram[nm][rows, :])
            t = pool.tile([P, D], DT_MM, tag=nm, name=f"{nm}_{i}")
            nc.gpsimd.tensor_copy(out=t, in_=stg)
            st[nm] = t
        S[i] = st

    for pre in range(min(2, ntiles)):
        stage_load(pre)
    load_weights(0)
    if ntiles:
        S[0]["xt1"] = transpose_xb(S[0]["x_loc"], "b1_0")

    for i in range(passes * ntiles + 1):
        a, b = i, i - 1
        ntot = passes * ntiles
        rows_a = slice((a % ntiles) * P, (a % ntiles + 1) * P)
        rows_b = slice((b % ntiles) * P, (b % ntiles + 1) * P)

        if a < ntot:
            S[a]["v1"] = matmul_block(0, S[a]["xt1"], S[a]["x_tac"], f"b1_{a}")  # x_tac is fp16
        if i == 0:
            load_weights(1)
        if b >= 0:
            S[b]["v3"] = matmul_block(2, S[b]["xt3"], S[b]["tac1_16"], f"b3_{b}")

        if a < ntot:
            xb2 = alloc_xb(f"b2_{a}")
            layernorm(S[a]["v1"], 1, None, xb_out=xb2)
            S[a]["tac1_16"] = xb2
            S[a]["xt2"] = transpose_xb(xb2, f"b2_{a}")
        if b >= 0:
            tac2 = actpool.tile([P, D], F32, tag="tac2", name=f"tac2_{b}")
            xb4 = alloc_xb(f"b4_{b}")
            layernorm(S[b]["v3"], 3, tac2, xb_out=xb4)
            S[b]["tac2"] = tac2
            S[b]["xt4"] = transpose_xb(xb4, f"b4_{b}")
            nc.gpsimd.dma_start(out=dram["o_tac"][rows_b, :], in_=tac2)

        if a < ntot:
            S[a]["v2"] = matmul_block(1, S[a]["xt2"], S[a]["x_str"], f"b2_{a}")
        if i == 0:
            load_weights(2)
            load_weights(3)
        if b >= 0:
            S[b]["v4"] = matmul_block(3, S[b]["xt4"], S[b]["x_loc"], f"b4_{b}")

        if a < ntot:
            strat = actpool.tile([P, D], F32, tag="strat", name=f"strat_{a}")
            xb3 = alloc_xb(f"b3_{a}")
            layernorm(S[a]["v2"], 2, strat, xb_out=xb3)
            S[a]["strat"] = strat
            S[a]["xt3"] = transpose_xb(xb3, f"b3_{a}")
            nc.gpsimd.dma_start(out=dram["o_str"][rows_a, :], in_=strat)
        if b >= 0:
            loc = outpool.tile([P, D], F32, tag="loc", name=f"loc_{b}")
            for h in range(HALVES):
                hs = slice(h * NHALF, (h + 1) * NHALF)
                nc.scalar.copy(out=loc[:, hs], in_=S[b]["v4"][h])
            nc.gpsimd.dma_start(out=dram["o_loc"][rows_b, :], in_=loc)
            del S[b]

        if i + 2 < ntot:
            stage_load(i + 2)
        if i + 1 < ntot:
            S[i + 1]["xt1"] = transpose_xb(S[i + 1]["x_loc"], f"b1_{i + 1}")


def host_prep(inputs, rows_per_core=ROWS_PER_CORE, ncores=NCORES):
    """Slice/precombine weights on host, shard batch across cores."""
    f = np.float32
    per_core_common = {}
    for i, name in enumerate(("l2t", "t2s", "s2t", "t2l")):
        w_in = np.asarray(inputs[f"{name}_w_in"], f)
        b_in = np.asarray(inputs[f"{name}_b_in"], f)
        w_out = np.asarray(inputs[f"{name}_w_out"], f)
        b_out = np.asarray(inputs[f"{name}_b_out"], f)
        wv = w_in[2 * D :]  # [D, D]
        bv = b_in[2 * D :]  # [D]
        m = w_out @ wv  # [D, D]
        per_core_common[f"mt{i}"] = np.ascontiguousarray(m.T)
        per_core_common[f"c{i}"] = np.ascontiguousarray(w_out @ bv + b_out)
    for j in (1, 2, 3):
        per_core_common[f"g{j}"] = np.asarray(inputs[f"ln{j}_g"], f)
        per_core_common[f"b{j}"] = np.asarray(inputs[f"ln{j}_b"], f)

    loc = np.asarray(inputs["local_features"], f)
    tac = np.asarray(inputs["tactical_features"], f)
    stra = np.asarray(inputs["strategic_features"], f)
    in_maps = []
    for c in range(ncores):
        rows = slice(c * rows_per_core, (c + 1) * rows_per_core)
        m = dict(per_core_common)
        m["x_loc"] = np.ascontiguousarray(loc[rows])
        m["x_tac"] = np.ascontiguousarray(tac[rows])
        m["x_str"] = np.ascontiguousarray(stra[rows])
        in_maps.append(m)
    return in_maps


_CACHED = {}


def _get_compiled():
    if "nc" not in _CACHED:
        _CACHED["nc"] = build_kernel()
    return _CACHED["nc"]


def kernel(**inputs):
    nc = _get_compiled()
    in_maps = host_prep(inputs)
    res = run_bass_kernel_spmd(nc, in_maps, list(range(NCORES)))
    loc = np.concatenate([res.results[c]["o_loc"] for c in range(NCORES)], axis=0)
    tac = np.concatenate([res.results[c]["o_tac"] for c in range(NCORES)], axis=0)
    stra = np.concatenate([res.results[c]["o_str"] for c in range(NCORES)], axis=0)
    return (loc, tac, stra)
